# revision 26
# baseline (speedup 1.0000x reference)
"""2-layer LSTM (B=128, T=256, D=512, H=1024) + linear head + ELU on 8 trn2 cores.

Strategy (all hardcoded; v11, 8.68ms vs 10.0ms v6 baseline):
  - Feature-major compute: gates^T [1024, B], h^T [H, B], c^T [H_local, B].
    Full batch B=128 as the matmul moving dim, weights as the 128x128
    stationary operand -> full PE utilization, no transposes anywhere.
  - Sharding: dies fully redundant; 4-way tensor-parallel over the hidden dim
    within a die. Core s owns hidden slice [256s, 256s+256) = 1024 local gate
    rows, ordered [i_lo f_lo g_lo o_lo | i_hi f_hi g_hi o_hi] x 128 so each
    PSUM bank holds an i/f/g/o quartet.
  - Communication: ONE combined [h0_t | h1_{t-1}] broadcast per peer per step
    (a message is always 128 descriptors - one per partition - and each
    (dest,lane) pair streams at ~200ns/descriptor on only 2 of 16 lanes, so
    per-lane descriptor count is the transport floor; splitting h0/h1 doubles
    it and measures WORSE). Each peer rides its own SWDGE queue (FIFO
    delivery keeps cumulative rsem counting sound). Descriptor-gen preps are
    emitted EARLY (gpsimd runs them under the PE burst, their deferred source
    read binds at trigger time); the cheap triggers carry the explicit
    sync deps on this step's elementwise writes. rsem += 2 per delivery;
    every consumer in program step t gates at >= 2t.
  - PE executes in order, so emission is phase-ordered: all flight-
    independent work first (L0 x of step t+1 prefetched into its own psum
    pair, self slots of L0h/L1x/L1h), then the h0(t-1)-gated layer-0 slots
    (whose elementwise feeds the send), then layer 1. Exactly one psum
    start (first mm) / stop (last mm) per bank.
  - X^T streams in packed 4 steps per DMA. Layer 1 lags layer 0 by TWO
    steps: its x-inputs h0(t-2) rode an already-arrived message, so all
    48 gated L1-x matmuls join the flight-overlap phase and the send
    trigger fires ~3.5us earlier each step.
  - Gate biases ride the ACT sigmoid/tanh ops as per-partition bias vectors.
    All weights SBUF-resident (bf16); c state fp32.
"""

import sys
from contextlib import ExitStack

import ml_dtypes
import numpy as np

for _p in ("/opt/trn_rl_repo", "/root/.axon_site/_ro/trn_rl_repo"):
    if _p not in sys.path:
        sys.path.append(_p)

import concourse.bacc as bacc
import concourse.mybir as mybir
import concourse.tile as tile
from concourse.bass_utils import run_bass_kernel_spmd
from concourse.tile_rust import add_dep_helper

F32 = mybir.dt.float32
BF16 = mybir.dt.bfloat16
AF = mybir.ActivationFunctionType

P = 128
T = 256
D = 512
H = 1024
BR = 256
B = 128    # full batch on every core (dies are redundant)
HL = 256   # hidden units per core
NL = 1024  # local gate rows per core
NUM_CORES = 8


def _build(nc, n_steps):
    assert n_steps % 4 == 0
    xt_in = nc.dram_tensor("XT", [n_steps // 4, P, 2048], BF16,
                           kind="ExternalInput").ap()
    wx0_in = nc.dram_tensor("Wx0", [D, NL], BF16, kind="ExternalInput").ap()
    wh0_in = nc.dram_tensor("Wh0", [H, NL], BF16, kind="ExternalInput").ap()
    wx1_in = nc.dram_tensor("Wx1", [H, NL], BF16, kind="ExternalInput").ap()
    wh1_in = nc.dram_tensor("Wh1", [H, NL], BF16, kind="ExternalInput").ap()
    wbr_in = nc.dram_tensor("Wbr", [H, BR], BF16, kind="ExternalInput").ap()
    b0_in = nc.dram_tensor("b0p", [P, 8], F32, kind="ExternalInput").ap()
    b1_in = nc.dram_tensor("b1p", [P, 8], F32, kind="ExternalInput").ap()
    bbr_in = nc.dram_tensor("bbrp", [P, 2], F32, kind="ExternalInput").ap()
    y_out = nc.dram_tensor("y", [2, P, B], F32, kind="ExternalOutput").ap()

    sWx0 = nc.alloc_sbuf_tensor("sWx0", [P, 4, NL], BF16).ap()
    sWh0 = nc.alloc_sbuf_tensor("sWh0", [P, 8, NL], BF16).ap()
    sWx1 = nc.alloc_sbuf_tensor("sWx1", [P, 8, NL], BF16).ap()
    sWh1 = nc.alloc_sbuf_tensor("sWh1", [P, 8, NL], BF16).ap()
    sWbr = nc.alloc_sbuf_tensor("sWbr", [P, 8, BR], BF16).ap()
    sB0 = nc.alloc_sbuf_tensor("sB0", [P, 8], F32).ap()
    sB1 = nc.alloc_sbuf_tensor("sB1", [P, 8], F32).ap()
    sBbr = nc.alloc_sbuf_tensor("sBbr", [P, 2], F32).ap()

    # gather ring (stable address for remote writes): slot k holds
    # [h0_lo h0_hi h1_lo h1_hi] of core (self^k); slot 0 (self) doubles as
    # the send source.
    gath = nc.alloc_sbuf_tensor("gath", [P, 3, 16, B], BF16).ap()
    cst = [nc.alloc_sbuf_tensor(f"c{l}", [P, 2, B], F32).ap() for l in range(2)]

    rsems0 = [nc.alloc_semaphore(f"r0sem{k}") for k in range(3)]
    rsems1 = [nc.alloc_semaphore(f"r1sem{k}") for k in range(3)]
    lsems = [nc.alloc_semaphore(f"lsem{k}") for k in range(3)]

    patches = []

    def h0c(g, j):   # h0 chunk j view of a gather slot-major tile [P, 16, B]
        return g[:, 4 * (j // 2) + (j % 2)]

    def h1c(g, j):
        return g[:, 4 * (j // 2) + 2 + (j % 2)]

    with tile.TileContext(nc) as tc:
        barrier_nop = nc.gpsimd.nop(nofuse=True)

        for sb, src, nk in ((sWx0, wx0_in, 4), (sWh0, wh0_in, 8),
                            (sWx1, wx1_in, 8), (sWh1, wh1_in, 8)):
            v = src.rearrange("(k p) n -> k p n", p=P)
            for k in range(nk):
                nc.sync.dma_start(out=sb[:, k], in_=v[k])
        wbrv = wbr_in.rearrange("(k p) n -> k p n", p=P)
        for k in range(8):
            nc.sync.dma_start(out=sWbr[:, k], in_=wbrv[k])
        nc.sync.dma_start(out=sB0, in_=b0_in)
        nc.sync.dma_start(out=sB1, in_=b1_in)
        nc.sync.dma_start(out=sBbr, in_=bbr_in)
        nc.vector.memset(cst[0], 0.0)
        nc.vector.memset(cst[1], 0.0)
        # steps 0/1 send a not-yet-written self h1 region (layer 1 lags by
        # two steps); zero slots 0 and 1 - peers never write the self region.
        gmemset = nc.vector.memset(gath[:, 0, 2:4], 0.0)
        gmemset2 = nc.vector.memset(gath[:, 1, 2:4], 0.0)

        stack = ExitStack()
        ps_pool = stack.enter_context(tc.tile_pool(name="psum", bufs=6, space="PSUM"))
        xt_pool = stack.enter_context(tc.tile_pool(name="xtp", bufs=3))
        tmp_pool = stack.enter_context(tc.tile_pool(name="tmp", bufs=8))
        hd_pool = stack.enter_context(tc.tile_pool(name="hdp", bufs=2))
        warm_pool = stack.enter_context(
            tc.tile_pool(name="warm", bufs=1, space="PSUM"))
        wtile = warm_pool.tile([P, 512], F32, name="warm")
        barrier_chained = set()

        def emit_warm(n, after):
            """Keep-warm matmuls into a scratch psum bank: the PE drops to
            the 1.2GHz p-state ~immediately when idle and takes 3us of busy
            time to ramp back, so idling through the message flight makes the
            whole post-arrival burst run slow. These are anchored after phase
            A; the caller chains phase B behind the returned leader."""
            first = None
            for i in range(n):
                mm = nc.tensor.matmul(
                    wtile[:, 0:128], sWx0[:, 0, 0:128], sWx0[:, 1, 0:128],
                    start=True, stop=True, skip_group_check=True)
                if first is None:
                    first = mm
                    add_dep_helper(mm.ins, after.ins, sync=False,
                                   reason="warm after phase A")
                else:
                    add_dep_helper(mm.ins, first.ins, sync=False,
                                   reason="warm chain")
            return first

        def gated_mms(mms_args, wait, chain_to=None):
            """Emit matmuls; the first carries `wait` (runtime patch) unless
            chain_to is given, in which case everything (including the first)
            is order-chained to that instruction instead."""
            first = chain_to
            for out, lhsT, rhs, start, stop in mms_args:
                mm = nc.tensor.matmul(out, lhsT, rhs, start=start, stop=stop)
                if first is None:
                    first = mm
                    if wait is not None:
                        patches.append((mm, wait[0], wait[1]))
                else:
                    add_dep_helper(mm.ins, first.ins, sync=False,
                                   reason="mms chained after gated first")
            return first

        def out_ap(pss, j):
            return pss[j // 4][:, 128 * (j % 4) : 128 * (j % 4 + 1)]

        def emit_slot(pss, w, tiles, s, gate, start=False, stop=False):
            """One gather-slot's 16 matmuls (2 k-chunks x 8 j) for a layer.
            start/stop: exactly one per psum bank across the whole layer-step
            (a stop clears the whole 2KiB zero-region group - sim semantics,
            hardware ignores it)."""
            args = [(out_ap(pss, j), w[:, k, 128 * j : 128 * (j + 1)], tiles[k],
                     start and k == 2 * s and j % 4 == 0,
                     stop and k == 2 * s + 1 and j % 4 == 3)
                    for j in range(8) for k in (2 * s, 2 * s + 1)]
            return gated_mms(args, gate)

        def emit_x0(pss, xt, t, stop):
            args = [(out_ap(pss, j), sWx0[:, kx, 128 * j : 128 * (j + 1)],
                     xt[:, t % 4, kx],
                     kx == 0 and j % 4 == 0,
                     stop and kx == 3 and j % 4 == 3)
                    for j in range(8) for kx in range(4)]
            gated_mms(args, None)

        def elem_quartet(ps, half, sB, c, out_bf):
            nc.scalar.activation(ps[:, 0:128], ps[:, 0:128], AF.Sigmoid,
                                 bias=sB[:, 4 * half + 0 : 4 * half + 1])
            nc.scalar.activation(ps[:, 128:256], ps[:, 128:256], AF.Sigmoid,
                                 bias=sB[:, 4 * half + 1 : 4 * half + 2])
            gsb = tmp_pool.tile([P, B], F32, name="gsb")
            nc.scalar.activation(gsb, ps[:, 256:384], AF.Tanh,
                                 bias=sB[:, 4 * half + 2 : 4 * half + 3])
            nc.scalar.activation(ps[:, 384:512], ps[:, 384:512], AF.Sigmoid,
                                 bias=sB[:, 4 * half + 3 : 4 * half + 4])
            t1 = tmp_pool.tile([P, B], F32, name="t1")
            nc.vector.tensor_mul(t1, ps[:, 0:128], gsb)              # i * g
            t2 = tmp_pool.tile([P, B], F32, name="t2")
            nc.vector.tensor_mul(t2, ps[:, 128:256], c[:, half])     # f * c
            nc.vector.tensor_add(c[:, half], t1, t2)
            tcn = tmp_pool.tile([P, B], F32, name="tc")
            nc.scalar.activation(tcn, c[:, half], AF.Tanh)
            return nc.vector.tensor_mul(out_bf, ps[:, 384:512], tcn)  # o*tanh(c)

        # h0 message: gath[:, slot, 0:2] -> peer k's gath[:, slot, 4k:4k+2]
        # h1 message: gath[:, slot, 2:4] -> peer k's gath[:, slot, 4k+2:4k+4]
        # Both messages of peer k ride queue k-1: same-lane FIFO delivery keeps
        # the cumulative rsem counting sound (h0(t) strictly before h1(t-1)).
        # Early preps carry no data deps (reads defer to the trigger), so each
        # queue's prep/trigger alternation is chained explicitly - otherwise
        # Tile may reorder preps across steps and triggers fire wrong entries.
        last_q = [barrier_nop, barrier_nop, barrier_nop]

        def send_preps(slot):
            """Emit the 3 per-peer broadcast preps for the combined
            [h0_t | h1_{t-1}] message (descriptor generation only - the
            source read is deferred to the trigger)."""
            src = gath[:, slot, 0:4]
            for k in range(1, 4):
                rd = [None] * 8
                rd[k] = (0, k)
                prep = nc.gpsimd.remote_dma_broadcast(
                    gath[:, slot, 4 * k : 4 * k + 4], src,
                    rsems0[k - 1], lsems[k - 1], rdests=rd, queue_num=k - 1)
                add_dep_helper(prep.ins, last_q[k - 1].ins, sync=False,
                               reason="queue FIFO order")
                last_q[k - 1] = prep

        def send_trigs(deps):
            """Fire one trigger per queue; the preps bound their (deferred)
            source read at emission time - i.e. to the slot's PREVIOUS
            version - so the real data dependency on this step's elementwise
            writes must be attached here explicitly."""
            for k in range(1, 4):
                trig = nc.gpsimd.trigger_dma(count=None, queue_num=k - 1)
                add_dep_helper(trig.ins, last_q[k - 1].ins, sync=False,
                               reason="queue FIFO order")
                for d in deps:
                    add_dep_helper(trig.ins, d.ins, sync=True,
                                   reason="send after data written")
                last_q[k - 1] = trig

        # ---------------- main loop ----------------
        # rsem protocol per peer: each delivery +2 on its message sem.
        # After step tau: rsems0 = 2*(tau+1) (h0(tau)), rsems1 = 2*(tau+1)
        # (h1(tau-1)). Every consumer in program step t gates at >= 2t.
        #
        # PE executes in program order, so a gated matmul head-of-line blocks
        # everything behind it. Emission order per step therefore puts ALL
        # flight-independent work first (L0 x, self slots of L0h/L1x/L1h),
        # then the h0(t-1)-gated slots, then the (later-arriving) h1(t-2)-
        # gated slots.
        xt4 = None
        pss0_cur = None   # L0 psum quartets for step t, x-part filled at t-1
        for t in range(n_steps):
            if t == 0:   # later groups are prefetched by the prior iteration
                xt4 = xt_pool.tile([P, 4, 4, B], BF16, name="xt")
                nc.sync.dma_start(
                    out=xt4,
                    in_=xt_in[0].rearrange("p (s k b) -> p s k b", s=4, k=4))
            gslot = gath[:, t % 3]
            gprev = gath[:, (t - 1) % 3]
            gprev2 = gath[:, (t + 1) % 3]   # == slot (t-2)%3
            h0p = [h0c(gprev, j) for j in range(8)]
            h0p2 = [h0c(gprev2, j) for j in range(8)]   # h0(t-2) for lag-2 L1
            h1p = [h1c(gprev, j) for j in range(8)]

            # phase A: everything that does not need this step's arrivals.
            # Layer 1 lags TWO steps (computes time t-2): its x-inputs
            # h0(t-2) rode message(t-2), so ALL its x matmuls join phase A.
            # The next step's L0 x-projection is prefetched here too.
            if pss0_cur is None:   # t == 0
                pss0_cur = [ps_pool.tile([P, 512], F32, name="ps")
                            for _ in range(2)]
                emit_x0(pss0_cur, xt4, t, stop=(t == 0))
            pss0 = pss0_cur
            if t + 1 < n_steps:
                xt4n = xt4
                if (t + 1) % 4 == 0:
                    xt4n = xt_pool.tile([P, 4, 4, B], BF16, name="xt")
                    nc.sync.dma_start(
                        out=xt4n,
                        in_=xt_in[(t + 1) // 4].rearrange(
                            "p (s k b) -> p s k b", s=4, k=4))
                    xt4 = xt4n
                pss0_cur = [ps_pool.tile([P, 512], F32, name="ps")
                            for _ in range(2)]
                emit_x0(pss0_cur, xt4n, t + 1, stop=False)
            aleader = None
            if t >= 1:
                aleader = emit_slot(pss0, sWh0, h0p, 0, None)
            if t >= 2:
                pss1 = [ps_pool.tile([P, 512], F32, name="ps") for _ in range(2)]
                emit_slot(pss1, sWx1, h0p2, 0, None, start=True)
                for s in (1, 2, 3):
                    emit_slot(pss1, sWx1, h0p2, s, (rsems0[s - 1], 2 * (t - 1)),
                              stop=(t == 2 and s == 3))
                if t >= 3:
                    emit_slot(pss1, sWh1, h1p, 0, None)
            send_preps(t % 3)
            warm = emit_warm(40, aleader) if t >= 2 else None

            # phase B: gated on the step t-1 message, layer-0 slots first so
            # its elementwise finishes earliest.
            if t >= 1:
                for s in (1, 2, 3):
                    bl = emit_slot(pss0, sWh0, h0p, s, (rsems0[s - 1], 2 * t),
                                   stop=(s == 3))
                    if s == 1 and warm is not None:
                        add_dep_helper(bl.ins, warm.ins, sync=False,
                                       reason="gated phase after warm")
            muls0 = [elem_quartet(pss0[half], half, sB0, cst[0],
                                  gslot[:, half]) for half in range(2)]

            # phase C: layer 1 recurrence slots (h1(t-3) rode message(t-1))
            if t >= 2:
                if t >= 3:
                    for s in (1, 2, 3):
                        emit_slot(pss1, sWh1, h1p, s, (rsems0[s - 1], 2 * t),
                                  stop=(s == 3))
                muls1 = [elem_quartet(pss1[half], half, sB1, cst[1],
                                      gslot[:, 2 + half]) for half in range(2)]
                send_trigs(muls0 + muls1)
            elif t == 1:
                send_trigs(muls0 + [gmemset2])
            else:
                send_trigs(muls0 + [gmemset])

        # tail: layer 1 times T-2 and T-1 (program steps T and T+1)
        for tt in range(2):
            tn = n_steps + tt
            gprev = gath[:, (tn - 1) % 3]
            gprev2 = gath[:, (tn + 1) % 3]
            gslot = gath[:, tn % 3]
            h0p2 = [h0c(gprev2, j) for j in range(8)]
            h1p = [h1c(gprev, j) for j in range(8)]
            pss1 = [ps_pool.tile([P, 512], F32, name="ps") for _ in range(2)]
            emit_slot(pss1, sWx1, h0p2, 0, None, start=True)
            for s in (1, 2, 3):
                emit_slot(pss1, sWx1, h0p2, s, (rsems0[s - 1], 2 * (tn - 1)))
            emit_slot(pss1, sWh1, h1p, 0, None)
            send_preps(tn % 3)
            for s in (1, 2, 3):
                emit_slot(pss1, sWh1, h1p, s, (rsems0[s - 1], 2 * tn),
                          stop=(s == 3))
            muls1 = [elem_quartet(pss1[half], half, sB1, cst[1],
                                  gslot[:, 2 + half]) for half in range(2)]
            # tail messages only carry fresh h1; stale h0 region never read.
            send_trigs(muls1)
        tn = n_steps + 1

        # ---------------- head: y^T = ELU(Wbr @ h1_last + bbr) -------------
        # h1(T-1) slices arrive with the last tail message: rsems0 = 2*tn+2
        gl = gath[:, tn % 3]
        psh = ps_pool.tile([P, 512], F32, name="ps")
        for s in range(4):
            hargs = [(psh[:, 128 * jo : 128 * (jo + 1)],
                      sWbr[:, k, 128 * jo : 128 * (jo + 1)],
                      h1c(gl, k), k == 0 and jo == 0, k == 7 and jo == 1)
                     for jo in range(2) for k in (2 * s, 2 * s + 1)]
            gated_mms(hargs, (rsems0[s - 1], 2 * tn + 2) if s >= 1 else None)
        for jo in range(2):
            pc = psh[:, 128 * jo : 128 * (jo + 1)]
            xv = hd_pool.tile([P, B], F32, name="xv")
            nc.scalar.activation(xv, pc, AF.Identity, bias=sBbr[:, jo : jo + 1])
            rl = hd_pool.tile([P, B], F32, name="rl")
            nc.vector.tensor_scalar_max(rl, xv, 0.0)
            mn = hd_pool.tile([P, B], F32, name="mn")
            nc.vector.tensor_scalar_min(mn, xv, 0.0)
            ex = hd_pool.tile([P, B], F32, name="ex")
            nc.scalar.activation(ex, mn, AF.Exp)
            s1 = hd_pool.tile([P, B], F32, name="s1")
            nc.vector.tensor_add(s1, rl, ex)
            yv = hd_pool.tile([P, B], F32, name="yv")
            nc.vector.tensor_scalar_add(yv, s1, -1.0)
            nc.sync.dma_start(out=y_out[jo], in_=yv)
        stack.close()

    nc._bir_kernel_barrier_sem_replica_groups.append(set(range(NUM_CORES)))
    barrier_nop.wait_op(nc._bir_kernel_barrier_sem, nc.bir_kernel_barrier_sem_inc,
                        "sem-ge", check=False)
    for inst, sem, val in patches:
        if val > 0:
            inst.wait_op(sem, val, "sem-ge", check=False)
    return patches


def build_program(n_steps=T):
    nc = bacc.Bacc("TRN2", target_bir_lowering=False, debug=False,
                   num_devices=NUM_CORES, num_swdge_queues=3)
    _build(nc, n_steps)
    nc.compile()
    return nc


def prepare_inputs(X, W_ih0, W_hh0, b_ih0, b_hh0, W_ih1, W_hh1, b_ih1, b_hh1,
                   W_br, b_br, n_steps=T):
    X = np.asarray(X, np.float32)
    bf = ml_dtypes.bfloat16
    # X^T packed 4 steps per row-block: [T/4, p, (step, k, b)]
    XT = (X[:, :n_steps].transpose(1, 2, 0)         # [T, D, B]
          .reshape(n_steps // 4, 4, 4, P, B)        # [T4, s, k, p, b]
          .transpose(0, 3, 1, 2, 4)                 # [T4, p, s, k, b]
          .reshape(n_steps // 4, P, 2048))
    XT = np.ascontiguousarray(XT).astype(bf)
    maps4 = []
    for s in range(4):
        cols = np.concatenate(
            [g * H + np.arange(HL * s + P * h, HL * s + P * h + P)
             for h in range(2) for g in range(4)])
        perm = np.concatenate(
            [np.arange(HL * (s ^ k), HL * (s ^ k) + HL) for k in range(4)])

        def w(a):
            return np.ascontiguousarray(np.asarray(a, np.float32)).astype(bf)

        b0 = np.asarray(b_ih0 + b_hh0, np.float32)[cols]
        b1 = np.asarray(b_ih1 + b_hh1, np.float32)[cols]
        maps4.append({
            "XT": XT,
            "Wx0": w(np.asarray(W_ih0).T[:, cols]),
            "Wh0": w(np.asarray(W_hh0).T[perm][:, cols]),
            "Wx1": w(np.asarray(W_ih1).T[perm][:, cols]),
            "Wh1": w(np.asarray(W_hh1).T[perm][:, cols]),
            "Wbr": w(np.asarray(W_br).T[perm]),
            "b0p": np.ascontiguousarray(b0.reshape(8, P).T),
            "b1p": np.ascontiguousarray(b1.reshape(8, P).T),
            "bbrp": np.ascontiguousarray(
                np.asarray(b_br, np.float32).reshape(2, P).T),
        })
    return [maps4[r % 4] for r in range(NUM_CORES)]


def collect(results):
    return np.ascontiguousarray(
        results[0]["y"].reshape(BR, B).T).astype(np.float32)


_cached_nc = None


def kernel(**inputs):
    global _cached_nc
    if _cached_nc is None:
        _cached_nc = build_program(T)
    in_maps = prepare_inputs(**inputs, n_steps=T)
    res = run_bass_kernel_spmd(_cached_nc, in_maps, list(range(NUM_CORES)))
    return collect(res.results)
